# revision 2
# baseline (speedup 1.0000x reference)
"""Grouped-experts SwiGLU MoE kernel for 8 Trainium2 NeuronCores.

Problem: x[16384, 2048] routed to 64 experts (256 contiguous tokens each);
per expert e: out_e = (silu(x_e @ w1[e]) * (x_e @ w3[e])) @ w2[e].

Sharding: expert-parallel. Core c owns experts 8c..8c+7 and tokens
[2048c, 2048(c+1)); each core computes its token slice fully locally.

v4 changes vs v3 (374 us, PE-bound at ~91% occupancy):
  * All 16-bit tensors are fp16 instead of bf16 (same PE rate & bytes,
    11-bit mantissa) -> base error drops 0.44% -> 0.05%, freeing the
    error budget for:
  * w13 of the first NF8=5 experts per core stored as fp8 e3m4 (scale
    x128; x is pre-divided by 128 so no on-chip rescale), upcast on
    scalar+gpsimd to fp16 ahead of the matmuls.  Cuts w13 DMA 64->44
    MiB/core so DMA (~268 us) has slack vs the PE floor (~330 us):
    weight prefetch never stalls the PE and the first weight chunk
    lands in half the time.
  * Startup: warmup matmul count tuned down, activation tables
    preloaded, critical e0 DMAs lead every queue.

All compute fp16 (fp32 PSUM accumulate); PE floor ~328 us/core.
"""

import ml_dtypes
import numpy as np

import concourse.bacc as bacc
import concourse.mybir as mybir
from concourse.bass_utils import run_bass_kernel_spmd
from concourse.tile import TileContext

N_CORES = 8
E_PER_CORE = 8          # experts per core
NF8 = 5                 # experts per core with e3m4 w13 (rest fp16)
TOK_PER_E = 256         # tokens per expert
DIM = 2048
HID = 1024
P = 128
KT = DIM // P           # 16 k-tiles (contraction over dim)
KT2 = HID // P          # 8 k-tiles (contraction over hidden)
MT = HID // P           # 8 hidden m-tiles in stage 1
NCH = DIM // 512        # 4 output n-chunks of 512 in stage 2
KKC = KT // 2           # 8 w13 chunks of 2 k-tiles
N_WARM = 8              # warmup matmuls (clock ramp until first weights)
WSCALE = 128.0          # w13 stored x128, x stored /128

F32 = mybir.dt.float32
F16 = mybir.dt.float16
E3 = mybir.dt.float8e3
SILU = mybir.ActivationFunctionType.Silu
IDENT = mybir.ActivationFunctionType.Identity
MULT = mybir.AluOpType.mult
NPF16 = np.float16
NPE3 = ml_dtypes.float8_e3m4

_program_cache = {}


def _build_program():
    """Per-core Bass program. Same program for all 8 cores (SPMD)."""
    nc = bacc.Bacc("TRN2", target_bir_lowering=False, debug=False)

    # xt: row (e*P + p) holds (x_e/128)^T[k*128+p, :] for all k
    xt_d = nc.dram_tensor("xt", [E_PER_CORE * P, KT, TOK_PER_E], F16,
                          kind="ExternalInput")
    # w13 fp8 part: experts 0..NF8-1; row ((e*KKC + kk)*P + p) =
    # [w1|w3]*128 rows for k=2kk,2kk+1 in e3m4
    w8_d = nc.dram_tensor("w13q", [NF8 * KKC * P, 2, 2 * HID], E3,
                          kind="ExternalInput")
    # w13 fp16 part: experts NF8..7 (x128)
    wf_d = nc.dram_tensor("w13f", [(E_PER_CORE - NF8) * KKC * P, 2, 2 * HID],
                          F16, kind="ExternalInput")
    # w2: row ((e*NCH + n)*P + p) = w2[e, :, n-chunk] k2-tiles
    w2_d = nc.dram_tensor("w2p", [E_PER_CORE * NCH * P, KT2, 512], F16,
                          kind="ExternalInput")
    out_d = nc.dram_tensor("out", [E_PER_CORE * TOK_PER_E, DIM], F16,
                           kind="ExternalOutput")

    with TileContext(nc) as tc:
        with tc.tile_pool(name="xt", bufs=12) as xt_p, \
             tc.tile_pool(name="w8", bufs=4) as w8_p, \
             tc.tile_pool(name="wt", bufs=5) as wt_p, \
             tc.tile_pool(name="w2", bufs=6) as w2_p, \
             tc.tile_pool(name="hT", bufs=12) as hT_p, \
             tc.tile_pool(name="gs", bufs=4) as gs_p, \
             tc.tile_pool(name="osb", bufs=4) as osb_p, \
             tc.tile_pool(name="ps", bufs=8, space="PSUM") as ps_p:

            # ---- critical-path DMAs for expert 0 lead every queue ----
            xtc0 = xt_p.tile([P, 4, TOK_PER_E], F16, tag="xt")
            nc.scalar.dma_start(out=xtc0[:], in_=xt_d[0:P, 0:4, :])
            w8t0 = None
            wt0 = wt_p.tile([P, 2, 2 * HID], F16, tag="wt")
            if NF8 > 0:
                w8t0 = w8_p.tile([P, 2, 2 * HID], E3, tag="w8")
                nc.sync.dma_start(out=w8t0[:], in_=w8_d[0:P])
            else:
                nc.sync.dma_start(out=wt0[:], in_=wf_d[0:P])

            # Preload both activation tables so the first real upcast /
            # silu doesn't eat a 1.3us ACT_TABLE_LOAD on the critical
            # path. Runs on garbage SBUF data; result discarded.
            warm = xt_p.tile([P, 4, TOK_PER_E], F16, tag="xt")
            nc.vector.memset(warm, 0.0)
            tw = gs_p.tile([P, 256], F32, tag="gs")
            nc.scalar.activation(tw[:, 0:8], warm[:, 0, 0:8], IDENT)
            nc.scalar.activation(tw[:, 8:16], warm[:, 0, 0:8], SILU)

            # HAM warm-up: the PE clock-gate defaults to 1.2 GHz and only
            # reaches 2.4 GHz after ~3.4 us of sustained activity.  Run
            # dummy matmuls on a zeroed tile until the first real weight
            # chunk + upcast land (~12 us) so the real stream starts at
            # full clock.
            wps = ps_p.tile([P, 512], F32, tag="ps")
            for _ in range(N_WARM):
                nc.tensor.matmul(wps[:], lhsT=warm[:, 0, 0:P],
                                 rhs=warm[:, 0:2, :], start=True, stop=True,
                                 skip_group_check=True)

            for e in range(E_PER_CORE):
                tok0 = e * TOK_PER_E
                is8 = e < NF8

                # ---- load xT k-tiles, 4 chunks so matmuls start early ----
                xtc = []
                for c in range(4):
                    if e == 0 and c == 0:
                        xtc.append(xtc0)
                        continue
                    xa = xt_p.tile([P, 4, TOK_PER_E], F16, tag="xt")
                    xtc.append(xa)
                    src = xt_d[e * P:(e + 1) * P, 4 * c:4 * (c + 1), :]
                    nc.gpsimd.dma_start(out=xa[:], in_=src)

                # ---- stage 1: g/u accumulation over dim ----
                gu = [ps_p.tile([P, 512], F32, tag="ps", name=f"gu_e{e}_m{m}")
                      for m in range(MT)]
                for kk in range(KKC):
                    wrow0 = (e * KKC + kk) * P if not is8 else \
                        (e * KKC + kk) * P
                    if is8:
                        if e == 0 and kk == 0:
                            w8t = w8t0
                        else:
                            w8t = w8_p.tile([P, 2, 2 * HID], E3, tag="w8")
                            nc.sync.dma_start(out=w8t[:],
                                              in_=w8_d[wrow0:wrow0 + P])
                        wt = wt_p.tile([P, 2, 2 * HID], F16, tag="wt")
                        # upcast halves in parallel on scalar + gpsimd
                        nc.scalar.activation(wt[:, 0, :], w8t[:, 0, :],
                                             IDENT)
                        nc.gpsimd.tensor_copy(wt[:, 1, :], w8t[:, 1, :])
                    else:
                        frow0 = ((e - NF8) * KKC + kk) * P
                        if e == 0 and kk == 0:
                            wt = wt0
                        else:
                            wt = wt_p.tile([P, 2, 2 * HID], F16, tag="wt")
                            nc.sync.dma_start(out=wt[:],
                                              in_=wf_d[frow0:frow0 + P])
                    for half in range(2):
                        k = 2 * kk + half
                        # start=True clears has_written for the WHOLE bank so
                        # only the first matmul into each gu bank may set it.
                        for m in range(MT):
                            xk = xtc[k // 4][:, k % 4, :]
                            nc.tensor.matmul(
                                gu[m][:, 0:256],
                                lhsT=wt[:, half, m * P:(m + 1) * P],
                                rhs=xk, start=(k == 0),
                                stop=(k == KT - 1), skip_group_check=True)
                            nc.tensor.matmul(
                                gu[m][:, 256:512],
                                lhsT=wt[:, half, HID + m * P:HID + (m + 1) * P],
                                rhs=xk, start=False,
                                stop=(k == KT - 1), skip_group_check=True)

                # w2 DMA triggers issue before the silus so the scalar ring
                # starts them as soon as the previous expert's out drains.
                w2ts = []
                for n in range(NCH):
                    w2t = w2_p.tile([P, KT2, 512], F16, tag="w2")
                    w2ts.append(w2t)
                    wrow0 = (e * NCH + n) * P
                    nc.scalar.dma_start(out=w2t[:],
                                        in_=w2_d[wrow0:wrow0 + P])

                # ---- h^T = silu(g^T) * u^T ----
                hT = []
                for m in range(MT):
                    gs = gs_p.tile([P, 256], F32, tag="gs")
                    nc.scalar.activation(gs[:], gu[m][:, 0:256], SILU)
                    ht = hT_p.tile([P, 256], F16, tag="hT")
                    hT.append(ht)
                    nc.vector.tensor_tensor(ht[:], gs[:], gu[m][:, 256:512],
                                            MULT)

                # ---- stage 2: out = h @ w2, m2-major ----
                last = (e == E_PER_CORE - 1)
                for m2 in range(2):
                    osb = osb_p.tile([P, DIM], F16, tag="osb")
                    trow0 = tok0 + m2 * P
                    for n in range(NCH):
                        ops = ps_p.tile([P, 512], F32, tag="ps")
                        for k2 in range(KT2):
                            nc.tensor.matmul(
                                ops[:],
                                lhsT=hT[k2][:, m2 * P:(m2 + 1) * P],
                                rhs=w2ts[n][:, k2, :],
                                start=(k2 == 0), stop=(k2 == KT2 - 1))
                        nc.vector.tensor_copy(
                            osb[:, n * 512:(n + 1) * 512], ops[:])
                        if last and m2 == 1:
                            # stream the final half out per n-chunk so the
                            # kernel tail is one 256 KiB DMA, not 1 MiB
                            nc.scalar.dma_start(
                                out=out_d[trow0:trow0 + P,
                                          n * 512:(n + 1) * 512],
                                in_=osb[:, n * 512:(n + 1) * 512])
                    if not (last and m2 == 1):
                        nc.scalar.dma_start(out=out_d[trow0:trow0 + P, :],
                                            in_=osb[:])

    nc.compile()
    return nc


def _get_program():
    if "nc" not in _program_cache:
        _program_cache["nc"] = _build_program()
    return _program_cache["nc"]


def _prep_inputs(x, w1, w2, w3):
    """Host repack: fp16/e3m4 cast + contiguous-descriptor layouts."""
    x = np.asarray(x, dtype=np.float32)
    w1 = np.asarray(w1, dtype=np.float32)
    w2 = np.asarray(w2, dtype=np.float32)
    w3 = np.asarray(w3, dtype=np.float32)

    E = w1.shape[0]
    assert E == N_CORES * E_PER_CORE and x.shape == (E * TOK_PER_E, DIM)

    # xt[e, p, k, t] = x[e*256 + t, k*128 + p] / 128
    xt = np.ascontiguousarray(
        (x / WSCALE).reshape(E, TOK_PER_E, KT, P).transpose(0, 3, 2, 1)
    ).astype(NPF16)
    # w13[e, kk, p, half, :] = 128*[w1[e, (2kk+half)P+p, :] | w3[e, ...]]
    w13 = np.concatenate(
        [w1.reshape(E, KT, P, HID), w3.reshape(E, KT, P, HID)], axis=3)
    w13 = np.ascontiguousarray(
        w13.reshape(E, KKC, 2, P, 2 * HID).transpose(0, 1, 3, 2, 4)) * WSCALE
    # w2p[e, n, p, k2, c] = w2[e, k2*P + p, n*512 + c]
    w2p = np.ascontiguousarray(
        w2.reshape(E, KT2, P, NCH, 512).transpose(0, 3, 2, 1, 4)).astype(NPF16)

    in_maps = []
    for c in range(N_CORES):
        e0 = c * E_PER_CORE
        w13c = w13[e0:e0 + E_PER_CORE]
        w13q = np.clip(w13c[:NF8], -15.0, 15.0).astype(NPE3)
        w13f = w13c[NF8:].astype(NPF16)
        in_maps.append({
            "xt": xt[e0:e0 + E_PER_CORE].reshape(E_PER_CORE * P, KT,
                                                 TOK_PER_E),
            "w13q": w13q.reshape(NF8 * KKC * P, 2, 2 * HID),
            "w13f": w13f.reshape((E_PER_CORE - NF8) * KKC * P, 2, 2 * HID),
            "w2p": w2p[e0:e0 + E_PER_CORE].reshape(E_PER_CORE * NCH * P, KT2,
                                                   512),
        })
    return in_maps


def kernel(x, w1, w2, w3, num_local_tokens_per_expert=None, **_unused):
    in_maps = _prep_inputs(x, w1, w2, w3)
    nc = _get_program()
    res = run_bass_kernel_spmd(nc, in_maps, list(range(N_CORES)))
    return np.concatenate(
        [res.results[c]["out"].astype(np.float32) for c in range(N_CORES)],
        axis=0)


# revision 4
# speedup vs baseline: 1.2198x; 1.2198x over previous
"""Grouped-experts SwiGLU MoE kernel for 8 Trainium2 NeuronCores.

Problem: x[16384, 2048] routed to 64 experts (256 contiguous tokens each);
per expert e: out_e = (silu(x_e @ w1[e]) * (x_e @ w3[e])) @ w2[e].

Sharding: expert-parallel. Core c owns experts 8c..8c+7 and tokens
[2048c, 2048(c+1)); each core computes its token slice fully locally.

v4 changes vs v3 (374 us, PE-bound at ~91% occupancy):
  * All 16-bit tensors are fp16 instead of bf16 (same PE rate & bytes,
    11-bit mantissa) -> base error drops 0.44% -> 0.05%, freeing the
    error budget for:
  * w13 of the first NF8=5 experts per core stored as fp8 e3m4 (scale
    x128; x is pre-divided by 128 so no on-chip rescale), upcast on
    scalar+gpsimd to fp16 ahead of the matmuls.  Cuts w13 DMA 64->44
    MiB/core so DMA (~268 us) has slack vs the PE floor (~330 us):
    weight prefetch never stalls the PE and the first weight chunk
    lands in half the time.
  * Startup: warmup matmul count tuned down, activation tables
    preloaded, critical e0 DMAs lead every queue.

All compute fp16 (fp32 PSUM accumulate); PE floor ~328 us/core.
"""

import ml_dtypes
import numpy as np

import concourse.bacc as bacc
import concourse.mybir as mybir
from concourse.bass_utils import run_bass_kernel_spmd
from concourse.tile import TileContext

N_CORES = 8
E_PER_CORE = 8          # experts per core
NF8 = 5                 # experts per core with e3m4 w13 (rest fp16)
TOK_PER_E = 256         # tokens per expert
DIM = 2048
HID = 1024
P = 128
KT = DIM // P           # 16 k-tiles (contraction over dim)
KT2 = HID // P          # 8 k-tiles (contraction over hidden)
MT = HID // P           # 8 hidden m-tiles in stage 1
NCH = DIM // 512        # 4 output n-chunks of 512 in stage 2
KKC = KT // 2           # 8 w13 chunks of 2 k-tiles
N_WARM = 8              # warmup matmuls (clock ramp until first weights)
WSCALE = 128.0          # w13 stored x128, x stored /128

F32 = mybir.dt.float32
F16 = mybir.dt.float16
E3 = mybir.dt.float8e3
SILU = mybir.ActivationFunctionType.Silu
IDENT = mybir.ActivationFunctionType.Identity
MULT = mybir.AluOpType.mult
NPF16 = np.float16
NPE3 = ml_dtypes.float8_e3m4

_program_cache = {}


def _build_program():
    """Per-core Bass program. Same program for all 8 cores (SPMD)."""
    nc = bacc.Bacc("TRN2", target_bir_lowering=False, debug=False)

    # xt: row (e*P + p) holds (x_e/128)^T[k*128+p, :] for all k
    xt_d = nc.dram_tensor("xt", [E_PER_CORE * P, KT, TOK_PER_E], F16,
                          kind="ExternalInput")
    # w13 fp8 part: experts 0..NF8-1; row ((e*KKC + kk)*P + p) =
    # [w1|w3]*128 rows for k=2kk,2kk+1 in e3m4
    w8_d = nc.dram_tensor("w13q", [NF8 * KKC * P, 2, 2 * HID], E3,
                          kind="ExternalInput")
    # w13 fp16 part: experts NF8..7 (x128)
    wf_d = nc.dram_tensor("w13f", [(E_PER_CORE - NF8) * KKC * P, 2, 2 * HID],
                          F16, kind="ExternalInput")
    # w2: row ((e*NCH + n)*P + p) = w2[e, :, n-chunk] k2-tiles
    w2_d = nc.dram_tensor("w2p", [E_PER_CORE * NCH * P, KT2, 512], F16,
                          kind="ExternalInput")
    out_d = nc.dram_tensor("out", [E_PER_CORE * TOK_PER_E, DIM], F16,
                           kind="ExternalOutput")

    with TileContext(nc) as tc:
        with tc.tile_pool(name="xt", bufs=12) as xt_p, \
             tc.tile_pool(name="w8", bufs=4) as w8_p, \
             tc.tile_pool(name="wt", bufs=6) as wt_p, \
             tc.tile_pool(name="w2", bufs=6) as w2_p, \
             tc.tile_pool(name="hT", bufs=12) as hT_p, \
             tc.tile_pool(name="gs", bufs=4) as gs_p, \
             tc.tile_pool(name="osb", bufs=4) as osb_p, \
             tc.tile_pool(name="ps", bufs=8, space="PSUM") as ps_p:

            # ---- critical-path DMAs for expert 0 lead every queue ----
            xtc0 = xt_p.tile([P, 4, TOK_PER_E], F16, tag="xt")
            nc.scalar.dma_start(out=xtc0[:], in_=xt_d[0:P, 0:4, :])
            w8t0 = None
            wt0 = wt_p.tile([P, 2, 2 * HID], F16, tag="wt")
            if NF8 > 0:
                w8t0 = w8_p.tile([P, 2, 2 * HID], E3, tag="w8")
                nc.sync.dma_start(out=w8t0[:], in_=w8_d[0:P])
            else:
                nc.sync.dma_start(out=wt0[:], in_=wf_d[0:P])

            # Preload both activation tables so the first real upcast /
            # silu doesn't eat a 1.3us ACT_TABLE_LOAD on the critical
            # path. Runs on garbage SBUF data; result discarded.
            warm = xt_p.tile([P, 4, TOK_PER_E], F16, tag="xt")
            nc.vector.memset(warm, 0.0)
            tw = gs_p.tile([P, 256], F32, tag="gs")
            nc.scalar.activation(tw[:, 0:8], warm[:, 0, 0:8], IDENT)
            nc.scalar.activation(tw[:, 8:16], warm[:, 0, 0:8], SILU)

            # HAM warm-up: the PE clock-gate defaults to 1.2 GHz and only
            # reaches 2.4 GHz after ~3.4 us of sustained activity.  Run
            # dummy matmuls on a zeroed tile until the first real weight
            # chunk + upcast land (~12 us) so the real stream starts at
            # full clock.
            wps = ps_p.tile([P, 512], F32, tag="ps")
            for _ in range(N_WARM):
                nc.tensor.matmul(wps[:], lhsT=warm[:, 0, 0:P],
                                 rhs=warm[:, 0:2, :], start=True, stop=True,
                                 skip_group_check=True)

            for e in range(E_PER_CORE):
                tok0 = e * TOK_PER_E
                is8 = e < NF8

                # ---- load xT k-tiles, 4 chunks so matmuls start early ----
                xtc = []
                for c in range(4):
                    if e == 0 and c == 0:
                        xtc.append(xtc0)
                        continue
                    xa = xt_p.tile([P, 4, TOK_PER_E], F16, tag="xt")
                    xtc.append(xa)
                    src = xt_d[e * P:(e + 1) * P, 4 * c:4 * (c + 1), :]
                    nc.gpsimd.dma_start(out=xa[:], in_=src)

                # ---- stage 1: g/u accumulation over dim ----
                gu = [ps_p.tile([P, 512], F32, tag="ps", name=f"gu_e{e}_m{m}")
                      for m in range(MT)]
                for kk in range(KKC):
                    wrow0 = (e * KKC + kk) * P if not is8 else \
                        (e * KKC + kk) * P
                    if is8:
                        if e == 0 and kk == 0:
                            w8t = w8t0
                        else:
                            w8t = w8_p.tile([P, 2, 2 * HID], E3, tag="w8")
                            nc.sync.dma_start(out=w8t[:],
                                              in_=w8_d[wrow0:wrow0 + P])
                        wt = wt_p.tile([P, 2, 2 * HID], F16, tag="wt")
                        # upcast halves in parallel on scalar + vector
                        # (gpsimd CAST measured 7us per half vs ~2us here)
                        nc.scalar.activation(wt[:, 0, :], w8t[:, 0, :],
                                             IDENT)
                        nc.vector.tensor_copy(wt[:, 1, :], w8t[:, 1, :])
                    else:
                        frow0 = ((e - NF8) * KKC + kk) * P
                        if e == 0 and kk == 0:
                            wt = wt0
                        else:
                            wt = wt_p.tile([P, 2, 2 * HID], F16, tag="wt")
                            nc.sync.dma_start(out=wt[:],
                                              in_=wf_d[frow0:frow0 + P])
                    for half in range(2):
                        k = 2 * kk + half
                        # start=True clears has_written for the WHOLE bank so
                        # only the first matmul into each gu bank may set it.
                        for m in range(MT):
                            xk = xtc[k // 4][:, k % 4, :]
                            nc.tensor.matmul(
                                gu[m][:, 0:256],
                                lhsT=wt[:, half, m * P:(m + 1) * P],
                                rhs=xk, start=(k == 0),
                                stop=(k == KT - 1), skip_group_check=True)
                            nc.tensor.matmul(
                                gu[m][:, 256:512],
                                lhsT=wt[:, half, HID + m * P:HID + (m + 1) * P],
                                rhs=xk, start=False,
                                stop=(k == KT - 1), skip_group_check=True)

                # w2 DMA triggers issue before the silus so the scalar ring
                # starts them as soon as the previous expert's out drains.
                w2ts = []
                for n in range(NCH):
                    w2t = w2_p.tile([P, KT2, 512], F16, tag="w2")
                    w2ts.append(w2t)
                    wrow0 = (e * NCH + n) * P
                    nc.scalar.dma_start(out=w2t[:],
                                        in_=w2_d[wrow0:wrow0 + P])

                # ---- h^T = silu(g^T) * u^T ----
                hT = []
                for m in range(MT):
                    gs = gs_p.tile([P, 256], F32, tag="gs")
                    nc.scalar.activation(gs[:], gu[m][:, 0:256], SILU)
                    ht = hT_p.tile([P, 256], F16, tag="hT")
                    hT.append(ht)
                    nc.vector.tensor_tensor(ht[:], gs[:], gu[m][:, 256:512],
                                            MULT)

                # ---- stage 2: out = h @ w2, m2-major ----
                last = (e == E_PER_CORE - 1)
                for m2 in range(2):
                    osb = osb_p.tile([P, DIM], F16, tag="osb")
                    trow0 = tok0 + m2 * P
                    for n in range(NCH):
                        ops = ps_p.tile([P, 512], F32, tag="ps")
                        for k2 in range(KT2):
                            nc.tensor.matmul(
                                ops[:],
                                lhsT=hT[k2][:, m2 * P:(m2 + 1) * P],
                                rhs=w2ts[n][:, k2, :],
                                start=(k2 == 0), stop=(k2 == KT2 - 1))
                        nc.vector.tensor_copy(
                            osb[:, n * 512:(n + 1) * 512], ops[:])
                        if last and m2 == 1:
                            # stream the final half out per n-chunk so the
                            # kernel tail is one 256 KiB DMA, not 1 MiB
                            nc.scalar.dma_start(
                                out=out_d[trow0:trow0 + P,
                                          n * 512:(n + 1) * 512],
                                in_=osb[:, n * 512:(n + 1) * 512])
                    if not (last and m2 == 1):
                        nc.scalar.dma_start(out=out_d[trow0:trow0 + P, :],
                                            in_=osb[:])

    nc.compile()
    return nc


def _get_program():
    if "nc" not in _program_cache:
        _program_cache["nc"] = _build_program()
    return _program_cache["nc"]


def _prep_inputs(x, w1, w2, w3):
    """Host repack: fp16/e3m4 cast + contiguous-descriptor layouts."""
    x = np.asarray(x, dtype=np.float32)
    w1 = np.asarray(w1, dtype=np.float32)
    w2 = np.asarray(w2, dtype=np.float32)
    w3 = np.asarray(w3, dtype=np.float32)

    E = w1.shape[0]
    assert E == N_CORES * E_PER_CORE and x.shape == (E * TOK_PER_E, DIM)

    # xt[e, p, k, t] = x[e*256 + t, k*128 + p] / 128
    xt = np.ascontiguousarray(
        (x / WSCALE).reshape(E, TOK_PER_E, KT, P).transpose(0, 3, 2, 1)
    ).astype(NPF16)
    # w13[e, kk, p, half, :] = 128*[w1[e, (2kk+half)P+p, :] | w3[e, ...]]
    w13 = np.concatenate(
        [w1.reshape(E, KT, P, HID), w3.reshape(E, KT, P, HID)], axis=3)
    w13 = np.ascontiguousarray(
        w13.reshape(E, KKC, 2, P, 2 * HID).transpose(0, 1, 3, 2, 4)) * WSCALE
    # w2p[e, n, p, k2, c] = w2[e, k2*P + p, n*512 + c]
    w2p = np.ascontiguousarray(
        w2.reshape(E, KT2, P, NCH, 512).transpose(0, 3, 2, 1, 4)).astype(NPF16)

    in_maps = []
    for c in range(N_CORES):
        e0 = c * E_PER_CORE
        w13c = w13[e0:e0 + E_PER_CORE]
        w13q = np.clip(w13c[:NF8], -15.0, 15.0).astype(NPE3)
        w13f = w13c[NF8:].astype(NPF16)
        in_maps.append({
            "xt": xt[e0:e0 + E_PER_CORE].reshape(E_PER_CORE * P, KT,
                                                 TOK_PER_E),
            "w13q": w13q.reshape(NF8 * KKC * P, 2, 2 * HID),
            "w13f": w13f.reshape((E_PER_CORE - NF8) * KKC * P, 2, 2 * HID),
            "w2p": w2p[e0:e0 + E_PER_CORE].reshape(E_PER_CORE * NCH * P, KT2,
                                                   512),
        })
    return in_maps


def kernel(x, w1, w2, w3, num_local_tokens_per_expert=None, **_unused):
    in_maps = _prep_inputs(x, w1, w2, w3)
    nc = _get_program()
    res = run_bass_kernel_spmd(nc, in_maps, list(range(N_CORES)))
    return np.concatenate(
        [res.results[c]["out"].astype(np.float32) for c in range(N_CORES)],
        axis=0)
